# revision 16
# baseline (speedup 1.0000x reference)
"""Trainium2 Bass kernel for the DDF (dynamic-filter + ECA + BN) module.

Distribution: data-parallel over batch B=8 across 8 NeuronCores (one image
per core).  All parameters replicated.  BN batch stats are all-reduced
across cores (sync-BN semantics, matching the reference).

Per-core layout: channels on partitions (2 channel-tiles of 128 on a single
[128, 2, XBUF] buffer), pixels on the free dimension.  Only the CENTER
window buffer is sent from HBM; the column-shifted copies are derived
on-device with SBUF->SBUF DMAs plus strided zero-fills of the wrapped
columns (gpsimd).  The per-pixel filter generator (1x1 conv C -> C*9) is
permuted on the host to o' = k*256 + c so that each PE output m-tile is one
(tap k, channel-tile) pair.  ECA channel attention is folded into a second
copy of the projection weights (W_proj * attn per input channel), so the
channel branch rides the mm2 contraction; taps 7 and 8 join it too.  ECA
pooling uses scalar-engine activations with accum_out; the tiny eca conv
runs on gpsimd so the vector engine never stalls for it.  BN statistics are
taken directly from the mm2 PSUM tiles; the mm2 output is copied to SBUF by
the DMA engines (not compute).  Sums are exchanged with a single 2KB
AllReduce, preceded by a warmup AllReduce at kernel start.

Emission is software-pipelined: mm2 of chunk i is emitted after mm1 of
chunk i+1 so the tensor engine never waits on the vector-engine add tree.
"""

import os

import numpy as np
import ml_dtypes

import concourse.bass as bass
import concourse.mybir as mybir
import concourse.tile as tile
from concourse import bacc
from concourse.bass_utils import run_bass_kernel_spmd

B, C, H, W = 8, 256, 64, 64
KS = 3
HW = H * W                    # 4096
GUARD = W                     # zero guard rows (one image row) at each end
XBUF = GUARD + HW + GUARD     # 4224
NCORES = 8
CT = 2                        # channel tiles of 128
MT1 = KS * KS * CT            # 18 mm1 output m-tiles
BN_EPS = 1e-5
F32 = mybir.dt.float32
BF16 = mybir.dt.bfloat16
ROWS_PER_CHUNK = 16
NCHUNKS = H // ROWS_PER_CHUNK  # 4
CHUNK = ROWS_PER_CHUNK * W     # 1024 pixels per chunk per channel-tile
NH = CHUNK // 512              # 512-px matmul groups per chunk

AF = mybir.ActivationFunctionType
ALU = mybir.AluOpType
RG = [list(range(NCORES))]


def _emit(tc):
    nc = tc.nc

    xbp = nc.declare_dram_parameter("xb", [CT, 128, XBUF], BF16, isOutput=False)
    wf = nc.declare_dram_parameter("wf", [CT, 128, MT1 * 128], BF16, isOutput=False)
    # misc fp32 params packed: bfp[18] | weca[3] | gam[2] | bet[2]
    misc = nc.declare_dram_parameter("misc", [128, MT1 + 7], F32, isOutput=False)
    wp = nc.declare_dram_parameter("wp", [128, CT * C], BF16, isOutput=False)
    yout = nc.declare_dram_parameter("y", [CT, 128, HW], BF16, isOutput=True)

    with (
        tc.tile_pool(name="consts", bufs=1) as consts,
        tc.tile_pool(name="fps", bufs=3, space="PSUM") as fps,
        tc.tile_pool(name="yps", bufs=2, space="PSUM") as yps,
        tc.tile_pool(name="fsb", bufs=6) as fsb_pool,
        tc.tile_pool(name="prod", bufs=1) as prod_pool,
        tc.tile_pool(name="stage", bufs=4) as stage_pool,
        tc.tile_pool(name="dram", bufs=1, space="DRAM") as dram,
    ):
        # ---- resident tensors -------------------------------------------
        wf_sb = [consts.tile([128, MT1 * 128], BF16, tag=f"wf{kt}", name=f"wf{kt}")
                 for kt in range(CT)]
        wpb = consts.tile([128, CT, C], BF16, tag="wpb", name="wpb")
        wp_sb = [wpb[:, kt, :] for kt in range(CT)]
        weffb = consts.tile([128, CT, C], BF16, tag="weffb", name="weffb")
        weff = [weffb[:, kt, :] for kt in range(CT)]
        miscb = consts.tile([128, MT1 + 7], F32, tag="miscb", name="miscb")
        bfp_sb = miscb[:, 0:MT1]
        wecab = miscb[:, MT1 : MT1 + 3]
        gam_sb = miscb[:, MT1 + 3 : MT1 + 5]
        bet_sb = miscb[:, MT1 + 5 : MT1 + 7]
        # window buffers, [dj] 0=left-shifted, 1=center, 2=right-shifted
        xb3 = [consts.tile([128, CT, XBUF], BF16, tag=f"xb{d}", name=f"xb{d}")
               for d in range(KS)]
        y_sb = [consts.tile([128, HW], F32, tag=f"ysb{mt}", name=f"ysb{mt}")
                for mt in range(CT)]
        stats_sb = [
            consts.tile([128, NCHUNKS * NH, 6], F32, tag=f"st{mt}", name=f"st{mt}")
            for mt in range(CT)
        ]
        pscr = consts.tile([128, CHUNK], BF16, tag="pscr", name="pscr")
        pacc = consts.tile([128, CT, NCHUNKS], F32, tag="pacc", name="pacc")
        zb = consts.tile([128, 1], F32, tag="zb", name="zb")
        nc.vector.memset(zb[:], 0.0)

        # ---- collective warmup ------------------------------------------
        warm_in = dram.tile([128, 1], F32, tag="wi", name="wi")
        warm_out = dram.tile([128, 1], F32, tag="wo", name="wo",
                             addr_space="Shared")
        nc.sync.dma_start(out=warm_in[:], in_=zb[:])
        nc.gpsimd.collective_compute(
            "AllReduce", ALU.add, replica_groups=RG,
            ins=[warm_in[:].opt()], outs=[warm_out[:].opt()],
        )

        # ---- input DMAs: x + derived copies on sync, weights on scalar --
        # x in 2 halves per ct; shifted copies in 2 halves per d.
        HB = 2 * CHUNK
        for h in range(2):
            lo = 0 if h == 0 else GUARD + HB
            hi = GUARD + HB if h == 0 else XBUF
            for ct in range(CT):
                nc.sync.dma_start(out=xb3[1][:, ct, lo:hi], in_=xbp[ct, :, lo:hi])
            clo = GUARD + h * HB
            for d, off in ((0, -1), (2, 1)):
                nc.sync.dma_start(
                    out=xb3[d][:, :, clo : clo + HB],
                    in_=xb3[1][:, :, clo + off : clo + HB + off],
                )
        WFA = 6 * 128  # taps 0-2
        for kt in range(CT):
            nc.scalar.dma_start(out=wf_sb[kt][:, 0:WFA], in_=wf[kt, :, 0:WFA])
        nc.scalar.dma_start(out=miscb[:], in_=misc[:, :])
        for kt in range(CT):
            nc.scalar.dma_start(out=wf_sb[kt][:, WFA:], in_=wf[kt, :, WFA:])
        nc.scalar.dma_start(
            out=wpb.rearrange("p c x -> p (c x)"), in_=wp[:, :]
        )

        # guard zeros for the derived buffers (vector, head only)
        for d in (0, 2):
            nc.vector.memset(xb3[d][:, :, 0:GUARD], 0.0)
            nc.vector.memset(xb3[d][:, :, GUARD + HW : XBUF], 0.0)

        # wrapped-column fixes for the derived buffers (gpsimd)
        def wrapfix(h):
            lo = GUARD + h * 2 * CHUNK
            v0 = xb3[0][:, :, lo : lo + 2 * CHUNK].rearrange(
                "p c (r w) -> p c r w", w=W)
            nc.gpsimd.memset(v0[:, :, :, 0:1], 0.0)
            v2 = xb3[2][:, :, lo : lo + 2 * CHUNK].rearrange(
                "p c (r w) -> p c r w", w=W)
            nc.gpsimd.memset(v2[:, :, :, W - 1 : W], 0.0)

        for h in range(2):
            wrapfix(h)

        # ECA pooling for the first x half on the vector engine (head slack)
        for ci in range(2):
            lo = GUARD + ci * CHUNK
            nc.vector.tensor_reduce(
                out=pacc[:, :, ci : ci + 1],
                in_=xb3[1][:, :, lo : lo + CHUNK],
                axis=mybir.AxisListType.X,
                op=ALU.add,
            )

        # ---- ECA pooling (scalar accum) + combine (gpsimd) --------------
        def pool_piece(ci):
            lo = GUARD + ci * CHUNK
            for ct in range(CT):
                nc.scalar.activation(
                    out=pscr[:], in_=xb3[1][:, ct, lo : lo + CHUNK],
                    func=AF.Copy, accum_out=pacc[:, ct, ci : ci + 1],
                )

        pool2 = consts.tile([128, CT], F32, tag="pool2", name="pool2")
        shd = consts.tile([128, CT], F32, tag="shd", name="shd")
        shu = consts.tile([128, CT], F32, tag="shu", name="shu")
        eca1 = consts.tile([128, CT], F32, tag="eca1", name="eca1")
        eca2 = consts.tile([128, CT], F32, tag="eca2", name="eca2")
        attn = consts.tile([128, CT], F32, tag="attn", name="attn")

        def emit_eca_combine():
            # pool2 = sum over the 4 chunk partials (gpsimd, tiny)
            nc.gpsimd.tensor_tensor(
                out=pool2[:], in0=pacc[:, :, 0], in1=pacc[:, :, 1], op=ALU.add
            )
            nc.gpsimd.tensor_tensor(
                out=pool2[:], in0=pool2[:], in1=pacc[:, :, 2], op=ALU.add
            )
            nc.gpsimd.tensor_tensor(
                out=pool2[:], in0=pool2[:], in1=pacc[:, :, 3], op=ALU.add
            )
            nc.gpsimd.memset(shd[:], 0.0)
            nc.gpsimd.memset(shu[:], 0.0)
            for ct in range(CT):
                nc.gpsimd.dma_start(
                    out=shd[1:128, ct : ct + 1], in_=pool2[0:127, ct : ct + 1]
                )
                nc.gpsimd.dma_start(
                    out=shu[0:127, ct : ct + 1], in_=pool2[1:128, ct : ct + 1]
                )
            nc.gpsimd.dma_start(out=shd[0:1, 1:2], in_=pool2[127:128, 0:1])
            nc.gpsimd.dma_start(out=shu[127:128, 0:1], in_=pool2[0:1, 1:2])
            nc.vector.tensor_scalar(
                out=eca1, in0=shd[:], scalar1=wecab[:, 0:1], scalar2=None,
                op0=ALU.mult,
            )
            nc.vector.scalar_tensor_tensor(
                out=eca2, in0=pool2[:], scalar=wecab[:, 1:2], in1=eca1[:],
                op0=ALU.mult, op1=ALU.add,
            )
            nc.vector.scalar_tensor_tensor(
                out=eca1, in0=shu[:], scalar=wecab[:, 2:3], in1=eca2[:],
                op0=ALU.mult, op1=ALU.add,
            )

        # ---- main loop ---------------------------------------------------
        fused_t = [None] * NCHUNKS
        p8_t = [None] * NCHUNKS
        ypt_t = [None] * NCHUNKS
        coff = [GUARD + ci * CHUNK for ci in range(NCHUNKS)]

        FUSE_DVE = ()   # taps whose evict+bias+product run fused on DVE

        def emit_mm1_chunk(ci, hooks=None):
            r0 = ci * ROWS_PER_CHUNK
            prods = []
            for k in range(KS * KS):
                di, dj = divmod(k, KS)
                woff = GUARD + (r0 + di - 1) * W
                nbufs = 2 if k >= 8 else 1
                pr = prod_pool.tile([128, CT, CHUNK], BF16, tag=f"pr{k}",
                                    name=f"pr{k}", bufs=nbufs)
                fused_tap = k in FUSE_DVE
                fsb = None
                if not fused_tap:
                    fsb = fsb_pool.tile([128, CT, CHUNK], BF16, tag="fsb",
                                        name="fsb")
                for ct in range(CT):
                    mt = k * CT + ct
                    fp = fps.tile([128, CHUNK], F32, tag="fp", name="fp")
                    for kt in range(CT):
                        lhsT = wf_sb[kt][:, mt * 128 : (mt + 1) * 128]
                        for nh in range(NH):
                            rhs = xb3[1][:, kt,
                                         coff[ci] + nh * 512 : coff[ci] + (nh + 1) * 512]
                            nc.tensor.matmul(
                                fp[:, nh * 512 : (nh + 1) * 512],
                                lhsT,
                                rhs,
                                start=(kt == 0),
                                stop=(kt == CT - 1),
                            )
                    if fused_tap:
                        # (fp + bias) * window in one DVE op from PSUM
                        nc.vector.scalar_tensor_tensor(
                            out=pr[:, ct, :], in0=fp[:],
                            scalar=bfp_sb[:, mt : mt + 1],
                            in1=xb3[dj][:, ct, woff : woff + CHUNK],
                            op0=ALU.add, op1=ALU.mult,
                        )
                    else:
                        nc.scalar.activation(
                            out=fsb[:, ct, :], in_=fp[:], func=AF.Identity,
                            bias=bfp_sb[:, mt : mt + 1], scale=1.0,
                        )
                if hooks and k in hooks:
                    for fn in hooks[k]:
                        fn()
                if not fused_tap:
                    # tap product against the shifted window (both ct at once)
                    nc.vector.tensor_tensor(
                        out=pr[:],
                        in0=fsb[:],
                        in1=xb3[dj][:, :, woff : woff + CHUNK],
                        op=ALU.mult,
                    )
                prods.append(pr)
                # weave the add tree
                if k == 1:
                    nc.vector.tensor_add(prods[0][:], prods[0][:], prods[1][:])
                elif k == 3:
                    nc.vector.tensor_add(prods[2][:], prods[2][:], prods[3][:])
                    nc.vector.tensor_add(prods[0][:], prods[0][:], prods[2][:])
                elif k == 5:
                    nc.vector.tensor_add(prods[4][:], prods[4][:], prods[5][:])
                elif k == 7:
                    nc.vector.tensor_add(prods[6][:], prods[6][:], prods[7][:])
                    nc.vector.tensor_add(prods[4][:], prods[4][:], prods[6][:])
                    ft = prod_pool.tile([128, CT, CHUNK], BF16, tag="fused",
                                        name="fused", bufs=2)
                    nc.vector.tensor_add(ft[:], prods[0][:], prods[4][:])
                    fused_t[ci] = ft
            p8_t[ci] = prods[8]

        def emit_mm2_part(ci, mt2, nh, xc_first=False):
            # one [128,512] output tile: fused, p8, attn-scaled x.  For the
            # last chunk the x source goes first (it is ready immediately).
            yp = yps.tile([128, 512], F32, tag="yp", name="yp")
            srcs = [(fused_t[ci], wp_sb), (p8_t[ci], wp_sb), (None, weff)]
            if xc_first:
                srcs = srcs[::-1]
            ns = len(srcs)
            for si, (srct, wtab) in enumerate(srcs):
                for kt in range(CT):
                    lhsT2 = wtab[kt][:, mt2 * 128 : (mt2 + 1) * 128]
                    if srct is None:
                        rhs = xb3[1][:, kt,
                                     coff[ci] + nh * 512 : coff[ci] + (nh + 1) * 512]
                    else:
                        rhs = srct[:, kt, nh * 512 : (nh + 1) * 512]
                    nc.tensor.matmul(
                        yp[:],
                        lhsT2,
                        rhs,
                        start=(si == 0 and kt == 0),
                        stop=(si == ns - 1 and kt == CT - 1),
                    )
            if ypt_t[ci] is None:
                ypt_t[ci] = [[None] * NH for _ in range(CT)]
            ypt_t[ci][mt2][nh] = yp

        def emit_yev(ci, mt2):
            # mm2 PSUM -> y_sb (scalar engine)
            r0 = ci * ROWS_PER_CHUNK
            for nh in range(NH):
                src = ypt_t[ci][mt2][nh]
                dst = y_sb[mt2][:, r0 * W + nh * 512 : r0 * W + (nh + 1) * 512]
                nc.scalar.activation(out=dst, in_=src[:], func=AF.Copy)

        def emit_bn(ci, mt2):
            for nh in range(NH):
                nc.vector.bn_stats(
                    out=stats_sb[mt2][:, ci * NH + nh, :],
                    in_=ypt_t[ci][mt2][nh][:],
                )

        def emit_weff():
            for kt in range(CT):
                nc.vector.tensor_scalar(
                    out=weff[kt][:], in0=wp_sb[kt][:],
                    scalar1=attn[:, kt : kt + 1], scalar2=None, op0=ALU.mult,
                )

        for ci in range(NCHUNKS):
            if ci == 0:
                hooks = {
                    6: [lambda: pool_piece(2)],
                    8: [lambda: pool_piece(3)],
                }
                emit_mm1_chunk(0, hooks=hooks)
                emit_eca_combine()
                # sigmoid = 1/(1+exp(-x)) with Exp on scalar (same act table)
                nc.scalar.activation(out=eca2[:], in_=eca1[:], func=AF.Exp,
                                     bias=zb[:, 0:1], scale=-1.0)
                nc.vector.tensor_scalar(
                    out=attn, in0=eca2[:], scalar1=1.0, scalar2=None,
                    op0=ALU.add,
                )
                nc.vector.reciprocal(out=attn[:], in_=attn[:])
                emit_weff()
            else:
                cj = ci - 1
                hooks = {
                    2: [lambda cj=cj: emit_mm2_part(cj, 0, 0)],
                    4: [lambda cj=cj: emit_mm2_part(cj, 0, 1)],
                    5: [lambda cj=cj: emit_yev(cj, 0),
                        lambda cj=cj: emit_bn(cj, 0)],
                    6: [lambda cj=cj: emit_mm2_part(cj, 1, 0)],
                    8: [lambda cj=cj: emit_mm2_part(cj, 1, 1)],
                }
                emit_mm1_chunk(ci, hooks=hooks)
                emit_yev(cj, 1)
                emit_bn(cj, 1)

        c3 = NCHUNKS - 1
        for mt2 in range(CT):
            for nh in range(NH):
                emit_mm2_part(c3, mt2, nh, xc_first=True)
        for mt2 in range(CT):
            emit_yev(c3, mt2)
            emit_bn(c3, mt2)

        # ---- global BN stats via all-reduce -----------------------------
        ps = consts.tile([128, CT, 2], F32, tag="ps", name="ps")
        for mt2 in range(CT):
            mv = consts.tile([128, 2], F32, tag=f"mv{mt2}", name=f"mv{mt2}")
            nc.vector.bn_aggr(out=mv[:], in_=stats_sb[mt2][:])
            nc.vector.tensor_scalar(
                out=ps[:, mt2, 0:1], in0=mv[:, 0:1], scalar1=1.0, scalar2=None,
                op0=ALU.mult,
            )
            nc.vector.scalar_tensor_tensor(
                out=ps[:, mt2, 1:2], in0=mv[:, 0:1], scalar=mv[:, 0:1],
                in1=mv[:, 1:2], op0=ALU.mult, op1=ALU.add,
            )
        nc.vector.tensor_scalar(
            out=ps[:], in0=ps[:], scalar1=float(HW), scalar2=None, op0=ALU.mult
        )

        ps_b = dram.tile([128, CT * 2], F32, tag="psb", name="psb")
        gs_b = dram.tile([128, CT * 2], F32, tag="gsb", name="gsb",
                         addr_space="Shared")
        nc.scalar.dma_start(out=ps_b[:], in_=ps.rearrange("p m two -> p (m two)"))
        nc.gpsimd.collective_compute(
            "AllReduce", ALU.add, replica_groups=RG,
            ins=[ps_b[:].opt()], outs=[gs_b[:].opt()],
        )
        gs = consts.tile([128, CT, 2], F32, tag="gs", name="gs")
        nc.sync.dma_start(out=gs.rearrange("p m two -> p (m two)"), in_=gs_b[:])

        # ---- normalize and write out ------------------------------------
        minv = 1.0 / float(B * HW)
        mg = consts.tile([128, CT], F32, tag="mg", name="mg")
        vg = consts.tile([128, CT], F32, tag="vg", name="vg")
        rr = consts.tile([128, CT], F32, tag="rr", name="rr")
        tt = consts.tile([128, CT], F32, tag="tt", name="tt")
        ac = consts.tile([128, CT], F32, tag="ac", name="ac")
        bc = consts.tile([128, CT], F32, tag="bc", name="bc")
        nc.vector.tensor_scalar(
            out=mg[:], in0=gs[:, :, 0], scalar1=minv, scalar2=None, op0=ALU.mult
        )
        nc.vector.tensor_scalar(
            out=vg[:], in0=gs[:, :, 1], scalar1=minv, scalar2=None, op0=ALU.mult
        )
        nc.vector.tensor_tensor(out=tt[:], in0=mg[:], in1=mg[:], op=ALU.mult)
        nc.vector.tensor_tensor(out=vg[:], in0=vg[:], in1=tt[:], op=ALU.subtract)
        nc.vector.tensor_scalar(
            out=vg[:], in0=vg[:], scalar1=1.0, scalar2=BN_EPS,
            op0=ALU.mult, op1=ALU.add,
        )
        nc.scalar.activation(out=tt[:], in_=vg[:], func=AF.Sqrt,
                             bias=zb[:, 0:1], scale=1.0)
        nc.vector.reciprocal(out=rr[:], in_=tt[:])
        nc.vector.tensor_tensor(out=ac[:], in0=rr[:], in1=gam_sb[:], op=ALU.mult)
        nc.vector.tensor_tensor(out=bc[:], in0=mg[:], in1=ac[:], op=ALU.mult)
        nc.vector.tensor_tensor(out=bc[:], in0=bet_sb[:], in1=bc[:], op=ALU.subtract)

        NSL = 4
        SL = HW // NSL
        idx = 0
        for si in range(NSL):
            for mt2 in range(CT):
                sl = slice(si * SL, (si + 1) * SL)
                st = stage_pool.tile([128, SL], BF16, tag="st", name="st")
                if idx % 8 < 5:
                    nc.vector.tensor_scalar(
                        out=st[:], in0=y_sb[mt2][:, sl],
                        scalar1=ac[:, mt2 : mt2 + 1], scalar2=bc[:, mt2 : mt2 + 1],
                        op0=ALU.mult, op1=ALU.add,
                    )
                    nc.sync.dma_start(out=yout[mt2, :, sl], in_=st[:])
                else:
                    nc.scalar.activation(
                        out=st[:], in_=y_sb[mt2][:, sl],
                        func=AF.Identity, bias=bc[:, mt2 : mt2 + 1],
                        scale=ac[:, mt2 : mt2 + 1],
                    )
                    nc.scalar.dma_start(out=yout[mt2, :, sl], in_=st[:])
                idx += 1


_NC = None


def _build_nc(debug=False):
    nc = bacc.Bacc(
        "TRN2", target_bir_lowering=False, debug=debug, num_devices=NCORES
    )
    with tile.TileContext(nc, num_cores=NCORES) as tc:
        _emit(tc)
    nc.compile()
    return nc


def _get_nc():
    global _NC
    if _NC is None:
        _NC = _build_nc()
    return _NC


def _prep_in_maps(x, W_filter, b_filter, w_eca, W_proj, gamma, beta):
    bf = ml_dtypes.bfloat16
    x = np.asarray(x, np.float32)
    W_filter = np.asarray(W_filter, np.float32)
    b_filter = np.asarray(b_filter, np.float32)
    w_eca = np.asarray(w_eca, np.float32)
    W_proj = np.asarray(W_proj, np.float32)
    gamma = np.asarray(gamma, np.float32)
    beta = np.asarray(beta, np.float32)

    # center window buffer: [64 zeros][x flat][64 zeros]
    buf = np.zeros((B, C, XBUF), np.float32)
    buf[:, :, GUARD : GUARD + HW] = x.reshape(B, C, HW)
    xb_h = np.ascontiguousarray(buf.reshape(B, CT, 128, XBUF)).astype(bf)

    # permute mm1 weights: o' = k*256 + c  (original o = c*9 + k)
    wperm = W_filter.reshape(C, KS * KS, C).transpose(1, 0, 2).reshape(KS * KS * C, C)
    wf_h = np.ascontiguousarray(wperm.T.reshape(CT, 128, MT1 * 128)).astype(bf)
    bperm = b_filter.reshape(C, KS * KS).T.reshape(KS * KS * C)

    wp_h = np.ascontiguousarray(
        (0.5 * W_proj).T.reshape(CT, 128, C).transpose(1, 0, 2).reshape(128, CT * C)
    ).astype(bf)
    misc_h = np.zeros((128, MT1 + 7), np.float32)
    misc_h[:, 0:MT1] = bperm.reshape(MT1, 128).T
    misc_h[:, MT1 : MT1 + 3] = (w_eca / float(HW)).reshape(1, 3)
    misc_h[:, MT1 + 3 : MT1 + 5] = gamma.reshape(CT, 128).T
    misc_h[:, MT1 + 5 : MT1 + 7] = beta.reshape(CT, 128).T

    in_maps = []
    for i in range(B):
        m = {
            "xb": xb_h[i],
            "wf": wf_h,
            "misc": misc_h,
            "wp": wp_h,
        }
        in_maps.append(m)
    return in_maps


last_result = None


def kernel(x, W_filter, b_filter, w_eca, W_proj, b_proj, gamma, beta):
    """Full-input, full-output DDF module on 8 NeuronCores."""
    global last_result
    # b_proj is mathematically cancelled by the batch-norm; unused.
    in_maps = _prep_in_maps(x, W_filter, b_filter, w_eca, W_proj, gamma, beta)
    nc = _get_nc()
    trace = bool(int(os.environ.get("DDF_TRACE", "0")))
    res = run_bass_kernel_spmd(nc, in_maps, list(range(NCORES)), trace=trace)
    last_result = res
    out = np.stack(
        [
            np.asarray(res.results[i]["y"]).reshape(C, H, W).astype(np.float32)
            for i in range(B)
        ]
    )
    return out


# revision 17
# speedup vs baseline: 1.1162x; 1.1162x over previous
"""Trainium2 Bass kernel for the DDF (dynamic-filter + ECA + BN) module.

Distribution: data-parallel over batch B=8 across 8 NeuronCores (one image
per core).  All parameters replicated.  BN batch stats are all-reduced
across cores (sync-BN semantics, matching the reference).

Per-core layout: channels on partitions (2 channel-tiles of 128 on a single
[128, 2, XBUF] buffer), pixels on the free dimension.  Only the CENTER
window buffer is sent from HBM; the column-shifted copies are derived
on-device with SBUF->SBUF DMAs plus strided zero-fills of the wrapped
columns (gpsimd).  The per-pixel filter generator (1x1 conv C -> C*9) is
permuted on the host to o' = k*256 + c so that each PE output m-tile is one
(tap k, channel-tile) pair.  ECA channel attention is folded into a second
copy of the projection weights (W_proj * attn per input channel), so the
channel branch rides the mm2 contraction; taps 7 and 8 join it too.  ECA
pooling uses scalar-engine activations with accum_out; the tiny eca conv
runs on gpsimd so the vector engine never stalls for it.  BN statistics are
taken directly from the mm2 PSUM tiles; the mm2 output is copied to SBUF by
the DMA engines (not compute).  Sums are exchanged with a single 2KB
AllReduce, preceded by a warmup AllReduce at kernel start.

Emission is software-pipelined: mm2 of chunk i is emitted after mm1 of
chunk i+1 so the tensor engine never waits on the vector-engine add tree.
"""

import os

import numpy as np
import ml_dtypes

import concourse.bass as bass
import concourse.mybir as mybir
import concourse.tile as tile
from concourse import bacc
from concourse.bass_utils import run_bass_kernel_spmd

B, C, H, W = 8, 256, 64, 64
KS = 3
HW = H * W                    # 4096
GUARD = W                     # zero guard rows (one image row) at each end
XBUF = GUARD + HW + GUARD     # 4224
NCORES = 8
CT = 2                        # channel tiles of 128
MT1 = KS * KS * CT            # 18 mm1 output m-tiles
BN_EPS = 1e-5
F32 = mybir.dt.float32
BF16 = mybir.dt.bfloat16
ROWS_PER_CHUNK = 16
NCHUNKS = H // ROWS_PER_CHUNK  # 4
CHUNK = ROWS_PER_CHUNK * W     # 1024 pixels per chunk per channel-tile
NH = CHUNK // 512              # 512-px matmul groups per chunk

AF = mybir.ActivationFunctionType
ALU = mybir.AluOpType
RG = [list(range(NCORES))]


def _emit(tc):
    nc = tc.nc

    xbp = nc.declare_dram_parameter("xb", [CT, 128, XBUF], BF16, isOutput=False)
    wf = nc.declare_dram_parameter("wf", [CT, 128, MT1 * 128], BF16, isOutput=False)
    # misc fp32 params packed: bfp[18] | weca[3] | gam[2] | bet[2]
    misc = nc.declare_dram_parameter("misc", [128, MT1 + 7], F32, isOutput=False)
    wp = nc.declare_dram_parameter("wp", [128, CT * C], BF16, isOutput=False)
    yout = nc.declare_dram_parameter("y", [CT, 128, HW], BF16, isOutput=True)

    with (
        tc.tile_pool(name="consts", bufs=1) as consts,
        tc.tile_pool(name="fps", bufs=3, space="PSUM") as fps,
        tc.tile_pool(name="yps", bufs=2, space="PSUM") as yps,
        tc.tile_pool(name="fsb", bufs=6) as fsb_pool,
        tc.tile_pool(name="prod", bufs=1) as prod_pool,
        tc.tile_pool(name="stage", bufs=4) as stage_pool,
        tc.tile_pool(name="dram", bufs=1, space="DRAM") as dram,
    ):
        # ---- resident tensors -------------------------------------------
        wf_sb = [consts.tile([128, MT1 * 128], BF16, tag=f"wf{kt}", name=f"wf{kt}")
                 for kt in range(CT)]
        wpb = consts.tile([128, CT, C], BF16, tag="wpb", name="wpb")
        wp_sb = [wpb[:, kt, :] for kt in range(CT)]
        weffb = consts.tile([128, CT, C], BF16, tag="weffb", name="weffb")
        weff = [weffb[:, kt, :] for kt in range(CT)]
        miscb = consts.tile([128, MT1 + 7], F32, tag="miscb", name="miscb")
        bfp_sb = miscb[:, 0:MT1]
        wecab = miscb[:, MT1 : MT1 + 3]
        gam_sb = miscb[:, MT1 + 3 : MT1 + 5]
        bet_sb = miscb[:, MT1 + 5 : MT1 + 7]
        # window buffers, [dj] 0=left-shifted, 1=center, 2=right-shifted
        xb3 = [consts.tile([128, CT, XBUF], BF16, tag=f"xb{d}", name=f"xb{d}")
               for d in range(KS)]
        y_sb = [consts.tile([128, HW], F32, tag=f"ysb{mt}", name=f"ysb{mt}")
                for mt in range(CT)]
        stats_sb = [
            consts.tile([128, NCHUNKS * NH, 6], F32, tag=f"st{mt}", name=f"st{mt}")
            for mt in range(CT)
        ]
        pscr = consts.tile([128, CHUNK], BF16, tag="pscr", name="pscr")
        pacc = consts.tile([128, CT, NCHUNKS], F32, tag="pacc", name="pacc")
        zb = consts.tile([128, 1], F32, tag="zb", name="zb")
        nc.vector.memset(zb[:], 0.0)

        # ---- collective warmup ------------------------------------------
        warm_in = dram.tile([128, 1], F32, tag="wi", name="wi")
        warm_out = dram.tile([128, 1], F32, tag="wo", name="wo",
                             addr_space="Shared")
        nc.gpsimd.dma_start(out=warm_in[:], in_=zb[:])
        nc.gpsimd.collective_compute(
            "AllReduce", ALU.add, replica_groups=RG,
            ins=[warm_in[:].opt()], outs=[warm_out[:].opt()],
        )

        # ---- input DMAs: x + derived copies on sync, weights on scalar --
        # x in 2 halves per ct; shifted copies in 2 halves per d.
        HB = 2 * CHUNK
        for h in range(2):
            lo = 0 if h == 0 else GUARD + HB
            hi = GUARD + HB if h == 0 else XBUF
            for ct in range(CT):
                nc.sync.dma_start(out=xb3[1][:, ct, lo:hi], in_=xbp[ct, :, lo:hi])
            clo = GUARD + h * HB
            for d, off in ((0, -1), (2, 1)):
                nc.sync.dma_start(
                    out=xb3[d][:, :, clo : clo + HB],
                    in_=xb3[1][:, :, clo + off : clo + HB + off],
                )
        for kt in range(CT):
            nc.scalar.dma_start(out=wf_sb[kt][:], in_=wf[kt])
        nc.scalar.dma_start(out=miscb[:], in_=misc[:, :])
        nc.scalar.dma_start(
            out=wpb.rearrange("p c x -> p (c x)"), in_=wp[:, :]
        )

        # guard zeros for the derived buffers (vector, head only)
        for d in (0, 2):
            nc.vector.memset(xb3[d][:, :, 0:GUARD], 0.0)
            nc.vector.memset(xb3[d][:, :, GUARD + HW : XBUF], 0.0)

        # wrapped-column fixes for the derived buffers (gpsimd)
        def wrapfix(h):
            lo = GUARD + h * 2 * CHUNK
            v0 = xb3[0][:, :, lo : lo + 2 * CHUNK].rearrange(
                "p c (r w) -> p c r w", w=W)
            nc.gpsimd.memset(v0[:, :, :, 0:1], 0.0)
            v2 = xb3[2][:, :, lo : lo + 2 * CHUNK].rearrange(
                "p c (r w) -> p c r w", w=W)
            nc.gpsimd.memset(v2[:, :, :, W - 1 : W], 0.0)

        for h in range(2):
            wrapfix(h)

        # ECA pooling for the first x half on the vector engine (head slack)
        for ci in range(2):
            lo = GUARD + ci * CHUNK
            nc.vector.tensor_reduce(
                out=pacc[:, :, ci : ci + 1],
                in_=xb3[1][:, :, lo : lo + CHUNK],
                axis=mybir.AxisListType.X,
                op=ALU.add,
            )

        # ---- ECA pooling (scalar accum) + combine (gpsimd) --------------
        def pool_piece(ci):
            lo = GUARD + ci * CHUNK
            for ct in range(CT):
                nc.scalar.activation(
                    out=pscr[:], in_=xb3[1][:, ct, lo : lo + CHUNK],
                    func=AF.Copy, accum_out=pacc[:, ct, ci : ci + 1],
                )

        pool2 = consts.tile([128, CT], F32, tag="pool2", name="pool2")
        shd = consts.tile([128, CT], F32, tag="shd", name="shd")
        shu = consts.tile([128, CT], F32, tag="shu", name="shu")
        eca1 = consts.tile([128, CT], F32, tag="eca1", name="eca1")
        eca2 = consts.tile([128, CT], F32, tag="eca2", name="eca2")
        attn = consts.tile([128, CT], F32, tag="attn", name="attn")

        def emit_eca_combine():
            # pool2 = sum over the 4 chunk partials (gpsimd, tiny)
            nc.gpsimd.tensor_tensor(
                out=pool2[:], in0=pacc[:, :, 0], in1=pacc[:, :, 1], op=ALU.add
            )
            nc.gpsimd.tensor_tensor(
                out=pool2[:], in0=pool2[:], in1=pacc[:, :, 2], op=ALU.add
            )
            nc.gpsimd.tensor_tensor(
                out=pool2[:], in0=pool2[:], in1=pacc[:, :, 3], op=ALU.add
            )
            nc.gpsimd.memset(shd[:], 0.0)
            nc.gpsimd.memset(shu[:], 0.0)
            for ct in range(CT):
                nc.gpsimd.dma_start(
                    out=shd[1:128, ct : ct + 1], in_=pool2[0:127, ct : ct + 1]
                )
                nc.gpsimd.dma_start(
                    out=shu[0:127, ct : ct + 1], in_=pool2[1:128, ct : ct + 1]
                )
            nc.gpsimd.dma_start(out=shd[0:1, 1:2], in_=pool2[127:128, 0:1])
            nc.gpsimd.dma_start(out=shu[127:128, 0:1], in_=pool2[0:1, 1:2])
            nc.vector.tensor_scalar(
                out=eca1, in0=shd[:], scalar1=wecab[:, 0:1], scalar2=None,
                op0=ALU.mult,
            )
            nc.vector.scalar_tensor_tensor(
                out=eca2, in0=pool2[:], scalar=wecab[:, 1:2], in1=eca1[:],
                op0=ALU.mult, op1=ALU.add,
            )
            nc.vector.scalar_tensor_tensor(
                out=eca1, in0=shu[:], scalar=wecab[:, 2:3], in1=eca2[:],
                op0=ALU.mult, op1=ALU.add,
            )

        # ---- main loop ---------------------------------------------------
        fused_t = [None] * NCHUNKS
        p8_t = [None] * NCHUNKS
        ypt_t = [None] * NCHUNKS
        coff = [GUARD + ci * CHUNK for ci in range(NCHUNKS)]

        FUSE_DVE = ()   # taps whose evict+bias+product run fused on DVE

        def emit_mm1_chunk(ci, hooks=None):
            r0 = ci * ROWS_PER_CHUNK
            prods = []
            for k in range(KS * KS):
                di, dj = divmod(k, KS)
                woff = GUARD + (r0 + di - 1) * W
                nbufs = 2 if k >= 8 else 1
                pr = prod_pool.tile([128, CT, CHUNK], BF16, tag=f"pr{k}",
                                    name=f"pr{k}", bufs=nbufs)
                fused_tap = k in FUSE_DVE
                fsb = None
                if not fused_tap:
                    fsb = fsb_pool.tile([128, CT, CHUNK], BF16, tag="fsb",
                                        name="fsb")
                for ct in range(CT):
                    mt = k * CT + ct
                    fp = fps.tile([128, CHUNK], F32, tag="fp", name="fp")
                    for kt in range(CT):
                        lhsT = wf_sb[kt][:, mt * 128 : (mt + 1) * 128]
                        for nh in range(NH):
                            rhs = xb3[1][:, kt,
                                         coff[ci] + nh * 512 : coff[ci] + (nh + 1) * 512]
                            nc.tensor.matmul(
                                fp[:, nh * 512 : (nh + 1) * 512],
                                lhsT,
                                rhs,
                                start=(kt == 0),
                                stop=(kt == CT - 1),
                            )
                    if fused_tap:
                        # (fp + bias) * window in one DVE op from PSUM
                        nc.vector.scalar_tensor_tensor(
                            out=pr[:, ct, :], in0=fp[:],
                            scalar=bfp_sb[:, mt : mt + 1],
                            in1=xb3[dj][:, ct, woff : woff + CHUNK],
                            op0=ALU.add, op1=ALU.mult,
                        )
                    else:
                        nc.scalar.activation(
                            out=fsb[:, ct, :], in_=fp[:], func=AF.Identity,
                            bias=bfp_sb[:, mt : mt + 1], scale=1.0,
                        )
                if hooks and k in hooks:
                    for fn in hooks[k]:
                        fn()
                if not fused_tap:
                    # tap product against the shifted window (both ct at once)
                    nc.vector.tensor_tensor(
                        out=pr[:],
                        in0=fsb[:],
                        in1=xb3[dj][:, :, woff : woff + CHUNK],
                        op=ALU.mult,
                    )
                prods.append(pr)
                # weave the add tree
                if k == 1:
                    nc.vector.tensor_add(prods[0][:], prods[0][:], prods[1][:])
                elif k == 3:
                    nc.vector.tensor_add(prods[2][:], prods[2][:], prods[3][:])
                    nc.vector.tensor_add(prods[0][:], prods[0][:], prods[2][:])
                elif k == 5:
                    nc.vector.tensor_add(prods[4][:], prods[4][:], prods[5][:])
                elif k == 7:
                    nc.vector.tensor_add(prods[6][:], prods[6][:], prods[7][:])
                    nc.vector.tensor_add(prods[4][:], prods[4][:], prods[6][:])
                    ft = prod_pool.tile([128, CT, CHUNK], BF16, tag="fused",
                                        name="fused", bufs=2)
                    nc.vector.tensor_add(ft[:], prods[0][:], prods[4][:])
                    fused_t[ci] = ft
            p8_t[ci] = prods[8]

        def emit_mm2_part(ci, mt2, nh, xc_first=False):
            # one [128,512] output tile: fused, p8, attn-scaled x.  For the
            # last chunk the x source goes first (it is ready immediately).
            yp = yps.tile([128, 512], F32, tag="yp", name="yp")
            srcs = [(fused_t[ci], wp_sb), (p8_t[ci], wp_sb), (None, weff)]
            if xc_first:
                srcs = srcs[::-1]
            ns = len(srcs)
            for si, (srct, wtab) in enumerate(srcs):
                for kt in range(CT):
                    lhsT2 = wtab[kt][:, mt2 * 128 : (mt2 + 1) * 128]
                    if srct is None:
                        rhs = xb3[1][:, kt,
                                     coff[ci] + nh * 512 : coff[ci] + (nh + 1) * 512]
                    else:
                        rhs = srct[:, kt, nh * 512 : (nh + 1) * 512]
                    nc.tensor.matmul(
                        yp[:],
                        lhsT2,
                        rhs,
                        start=(si == 0 and kt == 0),
                        stop=(si == ns - 1 and kt == CT - 1),
                    )
            if ypt_t[ci] is None:
                ypt_t[ci] = [[None] * NH for _ in range(CT)]
            ypt_t[ci][mt2][nh] = yp

        def emit_yev(ci, mt2):
            # mm2 PSUM -> y_sb (scalar engine)
            r0 = ci * ROWS_PER_CHUNK
            for nh in range(NH):
                src = ypt_t[ci][mt2][nh]
                dst = y_sb[mt2][:, r0 * W + nh * 512 : r0 * W + (nh + 1) * 512]
                nc.scalar.activation(out=dst, in_=src[:], func=AF.Copy)

        def emit_bn(ci, mt2):
            for nh in range(NH):
                nc.vector.bn_stats(
                    out=stats_sb[mt2][:, ci * NH + nh, :],
                    in_=ypt_t[ci][mt2][nh][:],
                )

        def emit_weff():
            for kt in range(CT):
                nc.vector.tensor_scalar(
                    out=weff[kt][:], in0=wp_sb[kt][:],
                    scalar1=attn[:, kt : kt + 1], scalar2=None, op0=ALU.mult,
                )

        for ci in range(NCHUNKS):
            if ci == 0:
                hooks = {
                    6: [lambda: pool_piece(2)],
                    8: [lambda: pool_piece(3)],
                }
                emit_mm1_chunk(0, hooks=hooks)
                emit_eca_combine()
                # sigmoid = 1/(1+exp(-x)) with Exp on scalar (same act table)
                nc.scalar.activation(out=eca2[:], in_=eca1[:], func=AF.Exp,
                                     bias=zb[:, 0:1], scale=-1.0)
                nc.vector.tensor_scalar(
                    out=attn, in0=eca2[:], scalar1=1.0, scalar2=None,
                    op0=ALU.add,
                )
                nc.vector.reciprocal(out=attn[:], in_=attn[:])
                emit_weff()
            else:
                cj = ci - 1
                hooks = {
                    2: [lambda cj=cj: emit_mm2_part(cj, 0, 0)],
                    4: [lambda cj=cj: emit_mm2_part(cj, 0, 1)],
                    5: [lambda cj=cj: emit_yev(cj, 0),
                        lambda cj=cj: emit_bn(cj, 0)],
                    6: [lambda cj=cj: emit_mm2_part(cj, 1, 0)],
                    8: [lambda cj=cj: emit_mm2_part(cj, 1, 1)],
                }
                emit_mm1_chunk(ci, hooks=hooks)
                emit_yev(cj, 1)
                emit_bn(cj, 1)

        c3 = NCHUNKS - 1
        for mt2 in range(CT):
            for nh in range(NH):
                emit_mm2_part(c3, mt2, nh, xc_first=True)
            emit_bn(c3, mt2)
            emit_yev(c3, mt2)

        # ---- global BN stats via all-reduce -----------------------------
        ps = consts.tile([128, CT, 2], F32, tag="ps", name="ps")
        for mt2 in range(CT):
            mv = consts.tile([128, 2], F32, tag=f"mv{mt2}", name=f"mv{mt2}")
            nc.vector.bn_aggr(out=mv[:], in_=stats_sb[mt2][:])
            nc.vector.tensor_scalar(
                out=ps[:, mt2, 0:1], in0=mv[:, 0:1], scalar1=1.0, scalar2=None,
                op0=ALU.mult,
            )
            nc.vector.scalar_tensor_tensor(
                out=ps[:, mt2, 1:2], in0=mv[:, 0:1], scalar=mv[:, 0:1],
                in1=mv[:, 1:2], op0=ALU.mult, op1=ALU.add,
            )
        nc.vector.tensor_scalar(
            out=ps[:], in0=ps[:], scalar1=float(HW), scalar2=None, op0=ALU.mult
        )

        ps_b = dram.tile([128, CT * 2], F32, tag="psb", name="psb")
        gs_b = dram.tile([128, CT * 2], F32, tag="gsb", name="gsb",
                         addr_space="Shared")
        nc.scalar.dma_start(out=ps_b[:], in_=ps.rearrange("p m two -> p (m two)"))
        nc.gpsimd.collective_compute(
            "AllReduce", ALU.add, replica_groups=RG,
            ins=[ps_b[:].opt()], outs=[gs_b[:].opt()],
        )
        gs = consts.tile([128, CT, 2], F32, tag="gs", name="gs")
        nc.sync.dma_start(out=gs.rearrange("p m two -> p (m two)"), in_=gs_b[:])

        # ---- normalize and write out ------------------------------------
        minv = 1.0 / float(B * HW)
        mg = consts.tile([128, CT], F32, tag="mg", name="mg")
        vg = consts.tile([128, CT], F32, tag="vg", name="vg")
        rr = consts.tile([128, CT], F32, tag="rr", name="rr")
        tt = consts.tile([128, CT], F32, tag="tt", name="tt")
        ac = consts.tile([128, CT], F32, tag="ac", name="ac")
        bc = consts.tile([128, CT], F32, tag="bc", name="bc")
        nc.vector.tensor_scalar(
            out=mg[:], in0=gs[:, :, 0], scalar1=minv, scalar2=None, op0=ALU.mult
        )
        nc.vector.tensor_scalar(
            out=vg[:], in0=gs[:, :, 1], scalar1=minv, scalar2=None, op0=ALU.mult
        )
        nc.vector.tensor_tensor(out=tt[:], in0=mg[:], in1=mg[:], op=ALU.mult)
        nc.vector.tensor_tensor(out=vg[:], in0=vg[:], in1=tt[:], op=ALU.subtract)
        nc.vector.tensor_scalar(
            out=vg[:], in0=vg[:], scalar1=1.0, scalar2=BN_EPS,
            op0=ALU.mult, op1=ALU.add,
        )
        nc.scalar.activation(out=tt[:], in_=vg[:], func=AF.Sqrt,
                             bias=zb[:, 0:1], scale=1.0)
        nc.vector.reciprocal(out=rr[:], in_=tt[:])
        nc.vector.tensor_tensor(out=ac[:], in0=rr[:], in1=gam_sb[:], op=ALU.mult)
        nc.vector.tensor_tensor(out=bc[:], in0=mg[:], in1=ac[:], op=ALU.mult)
        nc.vector.tensor_tensor(out=bc[:], in0=bet_sb[:], in1=bc[:], op=ALU.subtract)

        NSL = 4
        SL = HW // NSL
        idx = 0
        for si in range(NSL):
            for mt2 in range(CT):
                sl = slice(si * SL, (si + 1) * SL)
                st = stage_pool.tile([128, SL], BF16, tag="st", name="st")
                if idx % 8 < 5:
                    nc.vector.tensor_scalar(
                        out=st[:], in0=y_sb[mt2][:, sl],
                        scalar1=ac[:, mt2 : mt2 + 1], scalar2=bc[:, mt2 : mt2 + 1],
                        op0=ALU.mult, op1=ALU.add,
                    )
                    nc.sync.dma_start(out=yout[mt2, :, sl], in_=st[:])
                else:
                    nc.scalar.activation(
                        out=st[:], in_=y_sb[mt2][:, sl],
                        func=AF.Identity, bias=bc[:, mt2 : mt2 + 1],
                        scale=ac[:, mt2 : mt2 + 1],
                    )
                    nc.scalar.dma_start(out=yout[mt2, :, sl], in_=st[:])
                idx += 1


_NC = None


def _build_nc(debug=False):
    nc = bacc.Bacc(
        "TRN2", target_bir_lowering=False, debug=debug, num_devices=NCORES
    )
    with tile.TileContext(nc, num_cores=NCORES) as tc:
        _emit(tc)
    nc.compile()
    return nc


def _get_nc():
    global _NC
    if _NC is None:
        _NC = _build_nc()
    return _NC


def _prep_in_maps(x, W_filter, b_filter, w_eca, W_proj, gamma, beta):
    bf = ml_dtypes.bfloat16
    x = np.asarray(x, np.float32)
    W_filter = np.asarray(W_filter, np.float32)
    b_filter = np.asarray(b_filter, np.float32)
    w_eca = np.asarray(w_eca, np.float32)
    W_proj = np.asarray(W_proj, np.float32)
    gamma = np.asarray(gamma, np.float32)
    beta = np.asarray(beta, np.float32)

    # center window buffer: [64 zeros][x flat][64 zeros]
    buf = np.zeros((B, C, XBUF), np.float32)
    buf[:, :, GUARD : GUARD + HW] = x.reshape(B, C, HW)
    xb_h = np.ascontiguousarray(buf.reshape(B, CT, 128, XBUF)).astype(bf)

    # permute mm1 weights: o' = k*256 + c  (original o = c*9 + k)
    wperm = W_filter.reshape(C, KS * KS, C).transpose(1, 0, 2).reshape(KS * KS * C, C)
    wf_h = np.ascontiguousarray(wperm.T.reshape(CT, 128, MT1 * 128)).astype(bf)
    bperm = b_filter.reshape(C, KS * KS).T.reshape(KS * KS * C)

    wp_h = np.ascontiguousarray(
        (0.5 * W_proj).T.reshape(CT, 128, C).transpose(1, 0, 2).reshape(128, CT * C)
    ).astype(bf)
    misc_h = np.zeros((128, MT1 + 7), np.float32)
    misc_h[:, 0:MT1] = bperm.reshape(MT1, 128).T
    misc_h[:, MT1 : MT1 + 3] = (w_eca / float(HW)).reshape(1, 3)
    misc_h[:, MT1 + 3 : MT1 + 5] = gamma.reshape(CT, 128).T
    misc_h[:, MT1 + 5 : MT1 + 7] = beta.reshape(CT, 128).T

    in_maps = []
    for i in range(B):
        m = {
            "xb": xb_h[i],
            "wf": wf_h,
            "misc": misc_h,
            "wp": wp_h,
        }
        in_maps.append(m)
    return in_maps


last_result = None


def kernel(x, W_filter, b_filter, w_eca, W_proj, b_proj, gamma, beta):
    """Full-input, full-output DDF module on 8 NeuronCores."""
    global last_result
    # b_proj is mathematically cancelled by the batch-norm; unused.
    in_maps = _prep_in_maps(x, W_filter, b_filter, w_eca, W_proj, gamma, beta)
    nc = _get_nc()
    trace = bool(int(os.environ.get("DDF_TRACE", "0")))
    res = run_bass_kernel_spmd(nc, in_maps, list(range(NCORES)), trace=trace)
    last_result = res
    out = np.stack(
        [
            np.asarray(res.results[i]["y"]).reshape(C, H, W).astype(np.float32)
            for i in range(B)
        ]
    )
    return out
